# revision 6
# baseline (speedup 1.0000x reference)
"""Trainium2 Bass kernel: K-step Euler rollout of a kinematic bicycle model.

Full inputs:
  initial_state [131072, 4] f32, controls [131072, 64, 2] f32,
  timestep scalar f32, agents_pars [131072, 2] f32
Output: [131072, 64, 4] f32 (state after each of the 64 steps).

Strategy: pure data parallel over 8 NeuronCores (16384 agents each).
Per core the rollout is decomposed into 4 segmented prefix scans on DVE
(tensor_tensor_scan, 0/1 bf16 mask resets state at agent boundaries):
    V: dt*vel scan (65-slot bf16, slot0 = dt*vel0)
    W: yaw scan (65-slot bf16 inputs, fp32 scan state)
    X/Y: position scans writing fp32 out lanes directly.
tan(steer) ~= steer*(steer^2+3)/3 (|steer|<0.3), the /3 folded into
1/(3L).  cos(yaw) = sin(pi/2-|yaw|) (|yaw|<pi).  Engine balance:
DVE scans + cheap bf16 TS/TT (~9.8us/group), Scalar activations +
strided out-lane copies (~9.6), GpSimd big elementwise muls + tiny
seeds (~9.9).  Input DMAs ride the idle PE queue so the out-DMA
semaphore wait on the sync queue never delays prefetch; per-engine
issue order is readiness order so no engine stalls at its queue head.
"""
import os
import sys

for _p in ("/opt/trn_rl_repo", "/root/.axon_site/_ro/trn_rl_repo"):
    if os.path.isdir(_p) and _p not in sys.path:
        sys.path.insert(0, _p)

import numpy as np
import concourse.bass as bass
import concourse.bacc as bacc
import concourse.tile as tile
from concourse import mybir

F32 = mybir.dt.float32
BF16 = mybir.dt.bfloat16
AF = mybir.ActivationFunctionType
ALU = mybir.AluOpType

B = 131072
K = 64
NCORES = 8
BC = B // NCORES          # 16384 agents per core
P = 128                   # partitions
AG = 16                   # agents per partition per group
GRP = BC // (P * AG)      # 8 groups per core
PI = float(np.pi)

_cache = {}


def _build(dt: float):
    """Build the per-core SPMD program (identical on all 8 cores)."""
    nc = bacc.Bacc("TRN2", debug=False)

    d_aux = nc.dram_tensor("aux", [BC, 6], F32, kind="ExternalInput").ap()
    d_ctrl = nc.dram_tensor("controls", [BC, K, 2], F32, kind="ExternalInput").ap()
    d_out = nc.dram_tensor("out", [BC, K, 4], F32, kind="ExternalOutput").ap()

    r_aux = d_aux.rearrange("(g p a) c -> g p (a c)", g=GRP, p=P, a=AG)
    r_ctrl = d_ctrl.rearrange("(g p a) k c -> g p (a k c)", g=GRP, p=P, a=AG)
    r_out = d_out.rearrange("(g p a) k c -> g p (a k c)", g=GRP, p=P, a=AG)

    flat = lambda t: t.rearrange("p a k -> p (a k)")

    with tile.TileContext(nc) as tc:
        with (
            tc.tile_pool(name="consts", bufs=1) as consts,
            tc.tile_pool(name="io", bufs=2) as io,
            tc.tile_pool(name="mid", bufs=1) as mid,
        ):
            # constants
            mask65 = consts.tile([P, AG, 65], BF16)
            nc.vector.memset(mask65, 1.0)
            nc.vector.memset(mask65[:, :, 0], 0.0)
            mask64 = consts.tile([P, AG, 64], BF16)
            nc.vector.memset(mask64, 1.0)
            nc.vector.memset(mask64[:, :, 0], 0.0)
            c_dt2 = consts.tile([P, 1], F32)
            nc.vector.memset(c_dt2, dt * dt)
            c_pi2 = consts.tile([P, 1], F32)
            nc.vector.memset(c_pi2, PI / 2)
            c_m1 = consts.tile([P, 1], F32)
            nc.vector.memset(c_m1, -1.0)
            c_invdt = consts.tile([P, 1], F32)
            nc.vector.memset(c_invdt, 1.0 / dt)
            c_dtb = consts.tile([P, 1], F32)
            nc.vector.memset(c_dtb, dt)

            st = {}

            # ---- per-engine sub-stages; emitted in readiness order ----
            def s_load(g):
                ctrl_t = io.tile([P, AG, K, 2], F32, tag="ctrl", bufs=3, name=f"ctrl{g}")
                aux_t = io.tile([P, AG, 6], F32, tag="aux", bufs=8, name=f"aux{g}")
                nc.sync.dma_start(ctrl_t, r_ctrl[g])
                nc.sync.dma_start(aux_t, r_aux[g])
                st[g] = dict(ctrl=ctrl_t, aux=aux_t)

            def s_prep_g(g):  # GpSimd: t165 slot0 = dt*vel0
                d = st[g]
                t165 = mid.tile([P, AG, 65], BF16, tag="t165", bufs=7, name=f"t165_{g}")
                nc.gpsimd.tensor_tensor(
                    t165[:, :, 0], d["aux"][:, :, 3], c_dtb.broadcast_to([P, AG]), ALU.mult
                )
                d.update(t165=t165)

            def s_wmul_g(g):  # GpSimd: sL = steer/(3L); m1 = Vex*sL
                d = st[g]
                sL = mid.tile([P, AG, K], BF16, tag="sL", bufs=2, name=f"sL{g}")
                nc.gpsimd.tensor_tensor(
                    sL, d["ctrl"][:, :, :, 1],
                    d["invL3"].unsqueeze(2).broadcast_to([P, AG, K]), ALU.mult
                )
                m1 = mid.tile([P, AG, K], BF16, tag="m1", bufs=3, name=f"m1_{g}")
                nc.gpsimd.tensor_tensor(m1, d["t165"][:, :, 0:64], sL, ALU.mult)
                d.update(m1=m1)

            def s_scanxy_d(g):  # DVE: X/Y scans into out lanes
                d = st[g]
                out_t = io.tile([P, AG, K, 4], F32, tag="out", bufs=3, name=f"out{g}")
                d["out"] = out_t
                xlane = out_t[:, :, :, 0].rearrange("p a k -> p (a k)")
                nc.vector.tensor_tensor_scan(
                    xlane, flat(mask64), flat(d["xin"]), 0.0, ALU.mult, ALU.add
                )
                ylane = out_t[:, :, :, 1].rearrange("p a k -> p (a k)")
                nc.vector.tensor_tensor_scan(
                    ylane, flat(mask64), flat(d["yin"]), 0.0, ALU.mult, ALU.add
                )

            def s_scanxy_s(g):  # Scalar: yaw/vel out lanes
                d = st[g]
                out_t = d["out"]
                nc.scalar.activation(out_t[:, :, :, 2], d["w265"][:, :, 1:65], AF.Copy)
                nc.scalar.activation(out_t[:, :, :, 3], d["t165"][:, :, 1:65], AF.Copy, scale=c_invdt)

            def s_scanw_d(g):  # DVE: yaw scan (in-place, 65-slot)
                d = st[g]
                nc.vector.tensor_tensor_scan(
                    flat(d["w265"]), flat(mask65), flat(d["w265"]), 0.0, ALU.mult, ALU.add
                )

            def s_winw_d(g):  # DVE: w265[1:] = m1*q3p3
                d = st[g]
                w265 = mid.tile([P, AG, 65], BF16, tag="w265", bufs=6, name=f"w265_{g}")
                nc.vector.tensor_tensor(w265[:, :, 1:65], d["m1"], d["q3p3"], ALU.mult)
                d.update(w265=w265)

            def s_wmul_d(g):  # DVE: q3p3 = steer^2 + 3
                d = st[g]
                q3p3 = mid.tile([P, AG, K], BF16, tag="q3p3", bufs=3, name=f"q3p3_{g}")
                nc.vector.tensor_scalar_add(q3p3, d["q3"], 3.0)
                d.update(q3p3=q3p3)

            def s_prep_d(g):  # DVE: invL3 = 1/(3L); V scan
                d = st[g]
                t3L = mid.tile([P, AG], F32, tag="t3L", bufs=3, name=f"t3L{g}")
                nc.vector.tensor_scalar_mul(t3L, d["aux"][:, :, 4], 3.0)
                invL3 = mid.tile([P, AG], F32, tag="invL3", bufs=3, name=f"invL3_{g}")
                nc.vector.reciprocal(invL3, t3L)
                d.update(invL3=invL3)
                nc.vector.tensor_tensor_scan(
                    flat(d["t165"]), flat(mask65), flat(d["t165"]), 0.0, ALU.mult, ALU.add
                )

            def s_prep_s(g):  # Scalar: t165[1:] = dt^2*accel; q3 = steer^2
                d = st[g]
                nc.scalar.activation(d["t165"][:, :, 1:65], d["ctrl"][:, :, :, 0], AF.Copy, scale=c_dt2)
                q3 = mid.tile([P, AG, K], BF16, tag="q3", bufs=3, name=f"q3_{g}")
                nc.scalar.activation(q3, d["ctrl"][:, :, :, 1], AF.Square)
                d.update(q3=q3)

            def s_trig_s(g):  # Scalar: sinY, absY, cosY from Yex
                d = st[g]
                Yex = d["w265"][:, :, 0:64]
                sinY = mid.tile([P, AG, K], BF16, tag="sinY", bufs=3, name=f"sinY{g}")
                nc.scalar.activation(sinY, Yex, AF.Sin)
                absY = mid.tile([P, AG, K], BF16, tag="absY", bufs=2, name=f"absY{g}")
                nc.scalar.activation(absY, Yex, AF.Abs)
                cosY = mid.tile([P, AG, K], BF16, tag="cosY", bufs=3, name=f"cosY{g}")
                nc.scalar.activation(cosY, absY, AF.Sin, scale=c_m1, bias=c_pi2)
                d.update(sinY=sinY, cosY=cosY)

            def s_mulxy_g(g):  # GpSimd: xin/yin muls + seeds
                d = st[g]
                Vex = d["t165"][:, :, 0:64]
                xin = mid.tile([P, AG, K], F32, tag="xin", bufs=3, name=f"xin{g}")
                nc.gpsimd.tensor_tensor(xin, Vex, d["cosY"], ALU.mult)
                nc.gpsimd.tensor_tensor(xin[:, :, 0], xin[:, :, 0], d["aux"][:, :, 0], ALU.add)
                yin = mid.tile([P, AG, K], F32, tag="yin", bufs=3, name=f"yin{g}")
                nc.gpsimd.tensor_tensor(yin, Vex, d["sinY"], ALU.mult)
                nc.gpsimd.tensor_tensor(yin[:, :, 0], yin[:, :, 0], d["aux"][:, :, 1], ALU.add)
                d.update(xin=xin, yin=yin)

            def s_winw_g(g):  # GpSimd: w265 slot0 = yaw0 (emitted last: cross-eng dep)
                d = st[g]
                nc.gpsimd.tensor_copy(d["w265"][:, :, 0], d["aux"][:, :, 2])

            def s_dma(g):
                d = st.pop(g)
                nc.sync.dma_start(r_out[g], d["out"].rearrange("p a k c -> p (a k c)"))

            # (offset, fn): group for a stage at iteration i is i - offset.
            # List order = emission order = per-engine queue order.
            stages = [
                (0, s_load),      # sync queue: prefetch ctrl/aux (ahead of out-DMA wait)
                (1, s_prep_g),    # G: t165 slot0           (ready: aux @ i-1)
                (2, s_wmul_g),    # G: sL, m1               (ready: invL3 @ i-1)
                (7, s_scanxy_d),  # D: X/Y scans            (ready: xin/yin @ i-1)
                (7, s_scanxy_s),  # S: yaw/vel lanes        (ready: scans @ i-3/i-1)
                (4, s_scanw_d),   # D: W scan               (ready: w265 @ i-1)
                (3, s_winw_d),    # D: w265[1:] mul         (ready: m1 @ i-1)
                (2, s_wmul_d),    # D: q3p3                 (ready: q3 @ i-1)
                (1, s_prep_s),    # S: Aprep, q3            (ready: ctrl @ i-1)
                (1, s_prep_d),    # D: invL3, V scan        (ready: Aprep earlier this iter)
                (5, s_trig_s),    # S: trig                 (ready: Wscan @ i-1)
                (6, s_mulxy_g),   # G: xin/yin + seeds      (ready: trig @ i-1)
                (3, s_winw_g),    # G: w265 slot0           (ready: w265 alloc this iter, DVE pos 4)
                (7, s_dma),       # sync: out DMA
            ]
            for it in range(GRP + 8):
                for off, fn in stages:
                    g = it - off
                    if 0 <= g < GRP:
                        fn(g)

    nc.compile()
    return nc


def _get(dt: float):
    key = round(float(dt), 12)
    if key not in _cache:
        _cache[key] = _build(float(dt))
    return _cache[key]


def kernel(initial_state, controls, timestep, agents_pars, _trace=False):
    initial_state = np.ascontiguousarray(np.asarray(initial_state, dtype=np.float32))
    controls = np.ascontiguousarray(np.asarray(controls, dtype=np.float32))
    agents_pars = np.ascontiguousarray(np.asarray(agents_pars, dtype=np.float32))
    dt = float(np.asarray(timestep, dtype=np.float32))

    nc = _get(dt)
    aux = np.concatenate([initial_state, agents_pars], axis=1)
    in_maps = []
    for c in range(NCORES):
        s = slice(c * BC, (c + 1) * BC)
        in_maps.append({"aux": aux[s], "controls": controls[s]})
    from concourse import bass_utils

    r = bass_utils.run_bass_kernel_spmd(
        nc, in_maps, core_ids=list(range(NCORES)), trace=_trace
    )
    out = np.concatenate([r.results[c]["out"] for c in range(NCORES)], axis=0)
    if _trace:
        kernel.last_result = r
    return out


if __name__ == "__main__":
    # quick CoreSim check on one core's shard
    from concourse.bass_interp import CoreSim

    rng = np.random.default_rng(0)
    init = np.stack(
        [
            rng.normal(0, 10, BC),
            rng.normal(0, 10, BC),
            rng.normal(0, 0.5, BC),
            rng.normal(5, 2, BC),
        ],
        axis=-1,
    ).astype(np.float32)
    ctrl = (rng.standard_normal((BC, K, 2)) * np.array([1.0, 0.05])).astype(np.float32)
    pars = np.stack(
        [3 + 3 * rng.random(BC), 1.5 + rng.random(BC)], axis=-1
    ).astype(np.float32)
    dt = np.float32(0.1)

    nc = _get(float(dt))
    sim = CoreSim(nc, trace=False)
    sim.tensor("aux")[:] = np.concatenate([init, pars], axis=1)
    sim.tensor("controls")[:] = ctrl
    sim.simulate(check_with_hw=False)
    got = np.array(sim.tensor("out"))

    # numpy reference
    x, y, yaw, vel = (init[:, i].astype(np.float64) for i in range(4))
    L = pars[:, 0].astype(np.float64)
    exp = np.zeros((BC, K, 4))
    dtf = float(dt)
    for k in range(K):
        a = ctrl[:, k, 0].astype(np.float64)
        s = ctrl[:, k, 1].astype(np.float64)
        x = x + dtf * vel * np.cos(yaw)
        y = y + dtf * vel * np.sin(yaw)
        yaw = yaw + dtf * vel * np.tan(s) / L
        vel = vel + dtf * a
        exp[:, k] = np.stack([x, y, yaw, vel], axis=-1)
    err = np.linalg.norm(got - exp) / np.linalg.norm(exp)
    print("CoreSim relnorm vs numpy ref:", err)
    for c in range(4):
        e = np.abs(got[:, :, c] - exp[:, :, c]).max()
        print(f"  lane {c}: absmax {e:.3e}")
